# revision 44
# baseline (speedup 1.0000x reference)
"""BertCrf loss kernel for Trainium2 (8 NeuronCores, SPMD data-parallel).

Strategy
--------
Shapes: B=64, S=512, H=768, T=9 tags.  Loss = -sum_b(num_b - den_b).

The only heavy data is hidden_states [64,512,768] f32 (100 MB) -> the kernel
is memory-bound on streaming it once.  Each of the 8 cores takes 8 sequences.

Phase 1 (device, DMA-bound): emissions e^T [9, 4096] = fc_w @ h^T per core,
streamed from a host-pre-transposed hT [768, 4096] so the contraction dim
lands on partitions.  e^T (without fc_b; host adds it) is DMA'd back to the
host (147 KB/core) for the numerator.

Phase 2 (device): the CRF log-partition recurrence
  alpha_t[j] = logsumexp_k(alpha_{t-1,k} + trans[k,j]) + e_t[j]
is associative in the (log,+) semiring.  In linear space each step is
  P <- P @ (E * f_t[None,:]),  E = exp(trans), f_t = exp(e_t + fc_b - sigma),
so each length-8 chunk's product matrix is computed independently ->
8 seqs x 64 chunks = 512 independent 9x9 matrix chains, all advanced by a
SINGLE sequence of 8 steps: per step one block-diagonal [72,72]x[72,288]
bf16 matmul per half + one [72,288] VectorE scale per half (the two
halves are independent chains whose emission is interleaved so PE and DVE
ping-pong).  The constant shift sigma keeps fp32 in range (chunk
log-range ~ 13; fp32 overflows at 88).

Scan packing (hardcoded): chunk c = 8g + c3 with group g in [0,8),
c3 in [0,8); partitions (g, j) = 72; free index (c3, b, i) = 576.
Q[(g,j), (c3,b,i)] = P_{c,b}[i, j] (state, transposed per pair).
Because the token order is t-major (col = t*8 + b), PSUM bank g of the
emissions matmul (columns [512g, 512g+512)) is exactly group g's
(c3, s, b) factor panel: one [9,512] copy + one Exp builds all 512
chunks' scale factors.  Step s=0 is pure elementwise: Q_1 = Epat * F_0,
with Epat[(g,j),(c3,b,i)] = E[i,j] except 1.0 on chunk 0, whose F slot
holds exp(start_j + fc_b_j + e_0[b,j]) -> rows of P_0 all equal alpha_0.

DMA: 6.3MB/core (bf16) streamed as 512KB half-rows over three parallel
descriptor-generation paths (sync HWDGE, ACT HWDGE, gpsimd SWDGE),
half 0 first so bank 0-3 copies/F-gathers overlap half-1 streaming.

Host (cheap, exact f64): numerator from labels + e^T; combine the 64
chunk matrices per sequence (tiny 9x9 matvecs) with renormalization;
final logsumexp with end_transitions.  A full numpy fallback handles any
non-all-ones attention mask (the benchmark's mask is always ones).
"""

import numpy as np

# ---- problem constants (hardcoded per the task contract) ----
B, S, H, T = 64, 512, 768, 9
NCORES = 8
NB = B // NCORES          # 8 local sequences per core
NTOK = NB * S             # 4096 tokens per core
L = 8                     # chunk length (timesteps per chunk)
C = S // L                # 64 chunks
NG = 8                    # partition groups; chunk c = 8g + c3
C3 = 8                    # chunks per group
P_SCAN = NG * T           # 72 scan partitions
NFREE = C3 * NB * T       # 576 scan free columns (c3, b, i)
SIGMA = 0.8               # linear-space shift (range control)
KT = H // 128             # 6 contraction tiles

# token order is t-major: column index = t*NB + b.  Then PSUM bank g of the
# emissions matmul (columns [512g, 512g+512)) is exactly scan group g's
# (c3, s, b) panel: col = 512g + 64*c3 + 8*s + b.

MOVING_DTYPE = "bf16"     # "f32" | "f32r" | "bf16"  (hidden/fc_w matmul dtype)

_cached = {}


def _np_logsumexp(x, axis):
    m = np.max(x, axis=axis, keepdims=True)
    return (m + np.log(np.sum(np.exp(x - m), axis=axis, keepdims=True))).squeeze(axis)


def _reference_host(hidden_states, attention_mask, labels, fc_w, fc_b,
                    start_transitions, end_transitions, transitions):
    """Exact numpy port of the reference (f64) - fallback for unusual inputs."""
    e = (hidden_states.astype(np.float64) @ fc_w.T.astype(np.float64)) + fc_b
    mask = attention_mask.astype(bool)
    maskf = mask.astype(np.float64)
    labels = labels.astype(np.int64)
    b_idx = np.arange(e.shape[0])

    emit = np.take_along_axis(e, labels[..., None], axis=-1)[..., 0]
    trans_sc = transitions[labels[:, :-1], labels[:, 1:]].astype(np.float64)
    num = start_transitions[labels[:, 0]].astype(np.float64) + emit[:, 0]
    num = num + ((trans_sc + emit[:, 1:]) * maskf[:, 1:]).sum(1)
    last_idx = mask.astype(np.int64).sum(1) - 1
    num = num + end_transitions[labels[b_idx, last_idx]]

    alpha = start_transitions[None, :].astype(np.float64) + e[:, 0]
    for t in range(1, e.shape[1]):
        nxt = _np_logsumexp(alpha[:, :, None] + transitions[None].astype(np.float64)
                            + e[:, t][:, None, :], axis=1)
        alpha = np.where(mask[:, t][:, None], nxt, alpha)
    den = _np_logsumexp(alpha + end_transitions[None, :].astype(np.float64), axis=1)
    return np.float32(-(num - den).sum())


def _build_nc():
    """Build the per-core Bass program (same program on all 8 cores)."""
    import concourse.bacc as bacc
    import concourse.mybir as mybir
    import concourse.tile as tile

    dt = mybir.dt
    mdt = {"f32": dt.float32, "f32r": dt.float32, "bf16": dt.bfloat16}[MOVING_DTYPE]

    # Bacc (not raw Bass): its compile() pass legalizes multi-wait sync_info
    # into what this walrus build's per-instruction wait slots accept.
    nc = bacc.Bacc("TRN2", target_bir_lowering=False, debug=False)

    hT = nc.dram_tensor("hT", [H, NTOK], mdt, kind="ExternalInput")
    fcwT = nc.dram_tensor("fcwT", [H, T], mdt, kind="ExternalInput")
    lhsE = nc.dram_tensor("lhsE", [P_SCAN, P_SCAN], dt.bfloat16, kind="ExternalInput")
    epat = nc.dram_tensor("epat", [P_SCAN, NFREE], dt.float32, kind="ExternalInput")
    biasF = nc.dram_tensor("biasF", [P_SCAN, 1], dt.float32, kind="ExternalInput")
    bias0 = nc.dram_tensor("bias0", [T, 1], dt.float32, kind="ExternalInput")
    eT_out = nc.dram_tensor("eT_out", [T, NTOK], dt.float32, kind="ExternalOutput")
    q_out = nc.dram_tensor("q_out", [P_SCAN, NFREE], dt.float32,
                           kind="ExternalOutput")

    f32r = dt.float32r
    HALF = NFREE // 2          # 288 free columns per scan half-chain

    with tile.TileContext(nc) as tc:
        with (
            tc.tile_pool(name="const", bufs=1) as cpool,
            tc.tile_pool(name="hbuf", bufs=1) as hpool,
            tc.tile_pool(name="fbuf", bufs=1) as fpool,
            tc.tile_pool(name="scan", bufs=2) as qpool,
        ):
            # ---- phase 1 loads: 12 half-row 512KB DMAs on the two HWDGE
            # rings, HALF-MAJOR order: columns [0,2048) land first so banks
            # 0-3 finish (and their copies + F gathers run) while columns
            # [2048,4096) are still streaming ----
            hT_r = hT.rearrange("(kt p) n -> kt p n", p=128)
            fcw_sb = cpool.tile([128, KT, T], mdt)
            nc.gpsimd.dma_start(fcw_sb, fcwT.rearrange("(kt p) m -> p kt m", p=128))
            lhsE_sb = cpool.tile([P_SCAN, P_SCAN], dt.bfloat16)
            nc.gpsimd.dma_start(lhsE_sb, lhsE[:, :])
            epat_sb = cpool.tile([P_SCAN, NFREE], dt.float32)
            nc.gpsimd.dma_start(epat_sb, epat[:, :])
            biasF_sb = cpool.tile([P_SCAN, 1], dt.float32)
            nc.gpsimd.dma_start(biasF_sb, biasF[:, :])
            bias0_sb = cpool.tile([T, 1], dt.float32)
            nc.gpsimd.dma_start(bias0_sb, bias0[:, :])

            # kt0-4 halves go to the two HWDGE rings (h0 rows first on each
            # ring); kt5's halves ride the gpsimd SWDGE queue as a third
            # parallel path (it finishes its ~1MB before the rings do 2.5MB)
            HC = NTOK // 2
            htiles = [[None] * 2 for _ in range(KT)]
            for kt in range(KT):
                for h in range(2):
                    htiles[kt][h] = hpool.tile([128, HC], mdt,
                                               tag=f"ht{kt}_{h}",
                                               name=f"ht{kt}_{h}")
            issue = [(nc.sync, [(0, 0), (2, 0), (4, 0), (1, 1), (3, 1)]),
                     (nc.scalar, [(1, 0), (3, 0), (0, 1), (2, 1), (4, 1)]),
                     (nc.gpsimd, [(5, 0), (5, 1)])]
            for eng, lst in issue:
                for kt, h in lst:
                    eng.dma_start(htiles[kt][h],
                                  hT_r[kt, :, h * HC:(h + 1) * HC])

            f_raw = fpool.tile([P_SCAN, C3 * L * NB], dt.float32)
            eT_sb = fpool.tile([T, NTOK], dt.float32)
            with tc.tile_pool(name="psum1", bufs=1, space="PSUM") as pspool:
                psbank = [pspool.tile([T, 512], dt.float32, tag=f"psb{i}",
                                      name=f"psb{i}")
                          for i in range(8)]
                for h in range(2):
                    for kt in range(KT):
                        for nn in range(4):
                            n = 4 * h + nn
                            lw = fcw_sb[:, kt, :]
                            rh = htiles[kt][h][:, nn * 512:(nn + 1) * 512]
                            if MOVING_DTYPE == "f32r":
                                lw = lw.bitcast(f32r)
                                rh = rh.bitcast(f32r)
                            nc.tensor.matmul(psbank[n], lw, rh,
                                             start=(kt == 0), stop=(kt == KT - 1))
                    # banks of this half are done: PSUM->SBUF copy (DMA can't
                    # read PSUM; alternate DVE/ACT), then the F gather for
                    # scan group g on the SWDGE queue (rings are still
                    # streaming hT).  Column block [512g, 512(g+1)) of e^T is
                    # exactly group g's (c3, s, b) panel.
                    for nn in range(4):
                        g = 4 * h + nn
                        dst = eT_sb[:, g * 512:(g + 1) * 512]
                        if g % 2 == 0:
                            nc.vector.tensor_copy(dst, psbank[g])
                        else:
                            nc.scalar.copy(dst, psbank[g])
                        # h0 gathers on gpsimd (done with kt5 by then),
                        # h1 gathers on the sync ring (done with hT by then)
                        feng = nc.gpsimd if h == 0 else nc.sync
                        feng.dma_start(f_raw[g * T:(g + 1) * T], dst)

            # F_all[(g,j), (c3, s, b)] = exp(e - sigma + fcb) elementwise
            f_all = fpool.tile([P_SCAN, C3 * L * NB], dt.float32)
            nc.scalar.activation(f_all, f_raw,
                                 mybir.ActivationFunctionType.Exp, bias=biasF_sb)
            # chunk-0 s=0 slots: alpha_0 = exp(start_j + fcb_j + e_0[b,j])
            nc.scalar.activation(
                f_all[0:T, 0:NB], f_raw[0:T, 0:NB],
                mybir.ActivationFunctionType.Exp, bias=bias0_sb)
            f_v = f_all.rearrange("p (c3 s b) -> p c3 s b", c3=C3, s=L)

            # ---- scan: chunk c = 8g + c3, 8 steps, two independent
            # half-chains (c3 0-3 | 4-7) that interleave on PE/DVE ----
            def fslice(s, h):
                return f_v[:, 4 * h:4 * h + 4, s, :].unsqueeze(-1).broadcast_to(
                    [P_SCAN, 4, NB, T])

            # interleave the two chains' emission: engines run their program
            # IN ORDER, so A1,B1,A2,B2,... lets chain B's matmul fill the PE
            # while chain A's scale runs on the DVE (and vice versa)
            with tc.tile_pool(name="psq", bufs=4, space="PSUM") as psqpool:
                qcur = []
                for h in range(2):
                    q = qpool.tile([P_SCAN, 4, NB, T], dt.bfloat16, tag=f"q{h}",
                                   name=f"q{h}")
                    ep = epat_sb[:, h * HALF:(h + 1) * HALF]
                    nc.vector.tensor_mul(
                        q, ep.rearrange("p (c3 b i) -> p c3 b i", c3=4, b=NB),
                        fslice(0, h))
                    qcur.append(q)
                for s in range(1, L):
                    for h in range(2):
                        psq = psqpool.tile([P_SCAN, HALF], dt.float32, tag="psq",
                                           name="psq")
                        nc.tensor.matmul(
                            psq, lhsE_sb,
                            qcur[h].rearrange("p c3 b i -> p (c3 b i)"),
                            start=True, stop=True)
                        qn = qpool.tile([P_SCAN, 4, NB, T], dt.bfloat16,
                                        tag=f"q{h}", name=f"qn{h}")
                        nc.vector.tensor_mul(
                            qn, psq.rearrange("p (c3 b i) -> p c3 b i",
                                              c3=4, b=NB),
                            fslice(s, h))
                        qcur[h] = qn
                for h in range(2):
                    # back to f32 for the host
                    qff = fpool.tile([P_SCAN, HALF], dt.float32, tag=f"qf{h}",
                                     name=f"qf{h}")
                    nc.vector.tensor_copy(
                        qff, qcur[h].rearrange("p c3 b i -> p (c3 b i)"))
                    nc.scalar.dma_start(q_out[:, h * HALF:(h + 1) * HALF], qff)
            nc.scalar.dma_start(eT_out[:, :], eT_sb)

    nc.compile()
    return nc


def _get_nc():
    if "nc" not in _cached:
        _cached["nc"] = _build_nc()
    return _cached["nc"]


def _host_prep(hidden_states, fc_w, fc_b, start_transitions, transitions):
    """Build the 8 per-core input maps."""
    import ml_dtypes
    np_mdt = ml_dtypes.bfloat16 if MOVING_DTYPE == "bf16" else np.float32

    E = np.exp(transitions.astype(np.float64)).astype(np.float32)     # [T,T]
    # epat[(g,j),(c3,b,i)] = E[i,j], except chunk 0 (g=0, c3=0) slots = 1
    epat = np.tile(E.T[None, :, None, None, :], (NG, 1, C3, NB, 1))   # [g,j,c3,b,i]
    epat[0, :, 0, :, :] = 1.0
    epat = np.ascontiguousarray(epat.reshape(P_SCAN, NFREE), dtype=np.float32)
    # lhsE = blockdiag(E) x8: lhsT[(g,k),(g,j)] = E[k,j]  (bf16 scan matmul)
    lhsE = np.zeros((P_SCAN, P_SCAN), dtype=ml_dtypes.bfloat16)
    for g in range(NG):
        lhsE[g * T:(g + 1) * T, g * T:(g + 1) * T] = E.astype(ml_dtypes.bfloat16)
    fcwT = np.ascontiguousarray(fc_w.T.astype(np_mdt))                # [H,T]
    biasF = np.ascontiguousarray(
        np.tile(fc_b - SIGMA, NG).reshape(P_SCAN, 1), dtype=np.float32)
    bias0 = np.ascontiguousarray(
        (start_transitions + fc_b).reshape(T, 1), dtype=np.float32)

    in_maps = []
    for cid in range(NCORES):
        hc = hidden_states[cid * NB:(cid + 1) * NB]                   # [NB,S,H]
        # t-major token order: col = t*NB + b
        hc = hc.transpose(1, 0, 2).reshape(NTOK, H)
        hTc = np.ascontiguousarray(hc.T.astype(np_mdt))               # [H,4096]
        in_maps.append({
            "hT": hTc, "fcwT": fcwT, "lhsE": lhsE, "epat": epat,
            "biasF": biasF, "bias0": bias0,
        })
    return in_maps


def _host_finish(results, labels, fc_b, start_transitions,
                 end_transitions, transitions):
    """Numerator + chunk-matrix combine, all in f64."""
    labels = labels.astype(np.int64)
    start = start_transitions.astype(np.float64)
    end = end_transitions.astype(np.float64)
    trans = transitions.astype(np.float64)

    # reassemble e [B, S, T] from per-core e^T [9, 4096] (+ fc_b)
    # token order is t-major: col = t*NB + b
    e = np.empty((B, S, T), dtype=np.float64)
    for cid in range(NCORES):
        eT = results[cid]["eT_out"].astype(np.float64)    # [9, 4096]
        e[cid * NB:(cid + 1) * NB] = eT.T.reshape(S, NB, T).transpose(1, 0, 2)
    e += fc_b.astype(np.float64)

    # numerator (mask all-ones fast path)
    emit = np.take_along_axis(e, labels[..., None], axis=-1)[..., 0]
    num = start[labels[:, 0]] + emit[:, 0]
    num = num + (trans[labels[:, :-1], labels[:, 1:]] + emit[:, 1:]).sum(1)
    num = num + end[labels[:, -1]]

    # denominator: combine chunk matrices
    # chunk c = 8g + c3;  Q[(g,j), (c3,b,i)] = P_c[i, j]
    den = np.empty(B)
    for cid in range(NCORES):
        Q = results[cid]["q_out"].astype(np.float64)      # [72, 576]
        Q = Q.reshape(NG, T, C3, NB, T)                   # [g, j, c3, b, i]
        for b in range(NB):
            alpha = Q[0, :, 0, b, 0].copy()  # P_0[0,:] (rows of P_0 all equal)
            corr = 0.0
            for c in range(1, C):
                g, c3 = c // C3, c % C3
                Pc = Q[g, :, c3, b, :].T                  # P_c[i, j] rows i
                alpha = alpha @ Pc
                m = alpha.max()
                alpha /= m
                corr += np.log(m)
            den[cid * NB + b] = np.log((alpha * np.exp(end)).sum()) + corr \
                + (S - 1) * SIGMA
    return np.float32(-(num - den).sum())


def kernel(**inputs):
    hidden_states = np.asarray(inputs["hidden_states"], dtype=np.float32)
    attention_mask = np.asarray(inputs["attention_mask"])
    labels = np.asarray(inputs["labels"])
    fc_w = np.asarray(inputs["fc_w"], dtype=np.float32)
    fc_b = np.asarray(inputs["fc_b"], dtype=np.float32)
    start_transitions = np.asarray(inputs["start_transitions"], dtype=np.float32)
    end_transitions = np.asarray(inputs["end_transitions"], dtype=np.float32)
    transitions = np.asarray(inputs["transitions"], dtype=np.float32)

    if (hidden_states.shape != (B, S, H)) or not np.all(attention_mask != 0):
        return _reference_host(hidden_states, attention_mask, labels, fc_w,
                               fc_b, start_transitions, end_transitions,
                               transitions)

    from concourse.bass_utils import run_bass_kernel_spmd
    nc = _get_nc()
    in_maps = _host_prep(hidden_states, fc_w, fc_b, start_transitions,
                         transitions)
    res = run_bass_kernel_spmd(nc, in_maps, core_ids=list(range(NCORES)))
    _cached["last_res"] = res
    return _host_finish(res.results, labels, fc_b, start_transitions,
                        end_transitions, transitions)


# revision 46
# speedup vs baseline: 1.2963x; 1.2963x over previous
"""BertCrf loss kernel for Trainium2 (8 NeuronCores, SPMD data-parallel).

Strategy
--------
Shapes: B=64, S=512, H=768, T=9 tags.  Loss = -sum_b(num_b - den_b).

The only heavy data is hidden_states [64,512,768] f32 (100 MB) -> the kernel
is memory-bound on streaming it once.  Each of the 8 cores takes 8 sequences.

Phase 1 (device, DMA-bound): emissions e^T [9, 4096] = fc_w @ h^T per core,
streamed from a host-pre-transposed hT [768, 4096] so the contraction dim
lands on partitions.  e^T (without fc_b; host adds it) is DMA'd back to the
host (147 KB/core) for the numerator.

Phase 2 (device): the CRF log-partition recurrence
  alpha_t[j] = logsumexp_k(alpha_{t-1,k} + trans[k,j]) + e_t[j]
is associative in the (log,+) semiring.  In linear space each step is
  P <- P @ (E * f_t[None,:]),  E = exp(trans), f_t = exp(e_t + fc_b - sigma),
so each length-8 chunk's product matrix is computed independently ->
8 seqs x 64 chunks = 512 independent 9x9 matrix chains, all advanced by a
SINGLE sequence of 8 steps: per step one block-diagonal [72,72]x[72,288]
bf16 matmul per half + one [72,288] VectorE scale per half (the two
halves are independent chains whose emission is interleaved so PE and DVE
ping-pong).  The constant shift sigma keeps fp32 in range (chunk
log-range ~ 13; fp32 overflows at 88).

Scan packing (hardcoded): chunk c = 8g + c3 with group g in [0,8),
c3 in [0,8); partitions (g, j) = 72; free index (c3, b, i) = 576.
Q[(g,j), (c3,b,i)] = P_{c,b}[i, j] (state, transposed per pair).
Because the token order is t-major (col = t*8 + b), PSUM bank g of the
emissions matmul (columns [512g, 512g+512)) is exactly group g's
(c3, s, b) factor panel: one [9,512] copy + one Exp builds all 512
chunks' scale factors.  Step s=0 is pure elementwise: Q_1 = Epat * F_0,
with Epat[(g,j),(c3,b,i)] = E[i,j] except 1.0 on chunk 0, whose F slot
holds exp(start_j + fc_b_j + e_0[b,j]) -> rows of P_0 all equal alpha_0.

DMA: 6.3MB/core (bf16) streamed as 512KB half-rows over three parallel
descriptor-generation paths (sync HWDGE, ACT HWDGE, gpsimd SWDGE),
half 0 first so bank 0-3 copies/F-gathers overlap half-1 streaming.

Host (cheap, exact f64): numerator from labels + e^T; combine the 64
chunk matrices per sequence (tiny 9x9 matvecs) with renormalization;
final logsumexp with end_transitions.  A full numpy fallback handles any
non-all-ones attention mask (the benchmark's mask is always ones).
"""

import numpy as np

# ---- problem constants (hardcoded per the task contract) ----
B, S, H, T = 64, 512, 768, 9
NCORES = 8
NB = B // NCORES          # 8 local sequences per core
NTOK = NB * S             # 4096 tokens per core
L = 8                     # chunk length (timesteps per chunk)
C = S // L                # 64 chunks
NG = 8                    # partition groups; chunk c = 8g + c3
C3 = 8                    # chunks per group
P_SCAN = NG * T           # 72 scan partitions
NFREE = C3 * NB * T       # 576 scan free columns (c3, b, i)
SIGMA = 0.8               # linear-space shift (range control)
KT = H // 128             # 6 contraction tiles

# token order is t-major: column index = t*NB + b.  Then PSUM bank g of the
# emissions matmul (columns [512g, 512g+512)) is exactly scan group g's
# (c3, s, b) panel: col = 512g + 64*c3 + 8*s + b.

MOVING_DTYPE = "fp8"      # "f32" | "f32r" | "bf16" | "fp8"  (hidden/fc_w dtype)

_cached = {}


def _np_logsumexp(x, axis):
    m = np.max(x, axis=axis, keepdims=True)
    return (m + np.log(np.sum(np.exp(x - m), axis=axis, keepdims=True))).squeeze(axis)


def _reference_host(hidden_states, attention_mask, labels, fc_w, fc_b,
                    start_transitions, end_transitions, transitions):
    """Exact numpy port of the reference (f64) - fallback for unusual inputs."""
    e = (hidden_states.astype(np.float64) @ fc_w.T.astype(np.float64)) + fc_b
    mask = attention_mask.astype(bool)
    maskf = mask.astype(np.float64)
    labels = labels.astype(np.int64)
    b_idx = np.arange(e.shape[0])

    emit = np.take_along_axis(e, labels[..., None], axis=-1)[..., 0]
    trans_sc = transitions[labels[:, :-1], labels[:, 1:]].astype(np.float64)
    num = start_transitions[labels[:, 0]].astype(np.float64) + emit[:, 0]
    num = num + ((trans_sc + emit[:, 1:]) * maskf[:, 1:]).sum(1)
    last_idx = mask.astype(np.int64).sum(1) - 1
    num = num + end_transitions[labels[b_idx, last_idx]]

    alpha = start_transitions[None, :].astype(np.float64) + e[:, 0]
    for t in range(1, e.shape[1]):
        nxt = _np_logsumexp(alpha[:, :, None] + transitions[None].astype(np.float64)
                            + e[:, t][:, None, :], axis=1)
        alpha = np.where(mask[:, t][:, None], nxt, alpha)
    den = _np_logsumexp(alpha + end_transitions[None, :].astype(np.float64), axis=1)
    return np.float32(-(num - den).sum())


def _build_nc():
    """Build the per-core Bass program (same program on all 8 cores)."""
    import concourse.bacc as bacc
    import concourse.mybir as mybir
    import concourse.tile as tile

    dt = mybir.dt
    mdt = {"f32": dt.float32, "f32r": dt.float32, "bf16": dt.bfloat16,
           "fp8": dt.float8e4}[MOVING_DTYPE]

    # Bacc (not raw Bass): its compile() pass legalizes multi-wait sync_info
    # into what this walrus build's per-instruction wait slots accept.
    nc = bacc.Bacc("TRN2", target_bir_lowering=False, debug=False)

    hT = nc.dram_tensor("hT", [H, NTOK], mdt, kind="ExternalInput")
    fcwT = nc.dram_tensor("fcwT", [H, T], mdt, kind="ExternalInput")
    lhsE = nc.dram_tensor("lhsE", [P_SCAN, P_SCAN], dt.bfloat16, kind="ExternalInput")
    epat = nc.dram_tensor("epat", [P_SCAN, NFREE], dt.float32, kind="ExternalInput")
    biasF = nc.dram_tensor("biasF", [P_SCAN, 1], dt.float32, kind="ExternalInput")
    bias0 = nc.dram_tensor("bias0", [T, 1], dt.float32, kind="ExternalInput")
    eT_out = nc.dram_tensor("eT_out", [T, NTOK], dt.float32, kind="ExternalOutput")
    q_out = nc.dram_tensor("q_out", [P_SCAN, NFREE], dt.float32,
                           kind="ExternalOutput")

    f32r = dt.float32r
    HALF = NFREE // 2          # 288 free columns per scan half-chain

    with tile.TileContext(nc) as tc:
        with (
            tc.tile_pool(name="const", bufs=1) as cpool,
            tc.tile_pool(name="hbuf", bufs=1) as hpool,
            tc.tile_pool(name="fbuf", bufs=1) as fpool,
            tc.tile_pool(name="scan", bufs=2) as qpool,
        ):
            # ---- phase 1 loads: 12 half-row 512KB DMAs on the two HWDGE
            # rings, HALF-MAJOR order: columns [0,2048) land first so banks
            # 0-3 finish (and their copies + F gathers run) while columns
            # [2048,4096) are still streaming ----
            hT_r = hT.rearrange("(kt p) n -> kt p n", p=128)
            fcw_sb = cpool.tile([128, KT, T], mdt)
            nc.gpsimd.dma_start(fcw_sb, fcwT.rearrange("(kt p) m -> p kt m", p=128))
            lhsE_sb = cpool.tile([P_SCAN, P_SCAN], dt.bfloat16)
            nc.gpsimd.dma_start(lhsE_sb, lhsE[:, :])
            epat_sb = cpool.tile([P_SCAN, NFREE], dt.float32)
            nc.gpsimd.dma_start(epat_sb, epat[:, :])
            biasF_sb = cpool.tile([P_SCAN, 1], dt.float32)
            nc.gpsimd.dma_start(biasF_sb, biasF[:, :])
            bias0_sb = cpool.tile([T, 1], dt.float32)
            nc.gpsimd.dma_start(bias0_sb, bias0[:, :])

            # 12 half-row DMAs on the two HWDGE rings, HALF-MAJOR order:
            # columns [0,2048) land first so banks 0-3 finish (and their
            # copies + F gathers run) while columns [2048,4096) stream
            HC = NTOK // 2
            htiles = [[None] * 2 for _ in range(KT)]
            for h in range(2):
                for kt in range(KT):
                    ht = hpool.tile([128, HC], mdt, tag=f"ht{kt}_{h}",
                                    name=f"ht{kt}_{h}")
                    eng = nc.sync if kt % 2 == 0 else nc.scalar
                    eng.dma_start(ht, hT_r[kt, :, h * HC:(h + 1) * HC])
                    htiles[kt][h] = ht

            f_raw = fpool.tile([P_SCAN, C3 * L * NB], dt.float32)
            eT_sb = fpool.tile([T, NTOK], dt.float32)
            with tc.tile_pool(name="psum1", bufs=1, space="PSUM") as pspool:
                psbank = [pspool.tile([T, 512], dt.float32, tag=f"psb{i}",
                                      name=f"psb{i}")
                          for i in range(8)]
                for h in range(2):
                    for kt in range(KT):
                        for nn in range(4):
                            n = 4 * h + nn
                            lw = fcw_sb[:, kt, :]
                            rh = htiles[kt][h][:, nn * 512:(nn + 1) * 512]
                            if MOVING_DTYPE == "f32r":
                                lw = lw.bitcast(f32r)
                                rh = rh.bitcast(f32r)
                            nc.tensor.matmul(psbank[n], lw, rh,
                                             start=(kt == 0), stop=(kt == KT - 1))
                    # banks of this half are done: PSUM->SBUF copy (DMA can't
                    # read PSUM; alternate DVE/ACT), then the F gather for
                    # scan group g on the SWDGE queue (rings are still
                    # streaming hT).  Column block [512g, 512(g+1)) of e^T is
                    # exactly group g's (c3, s, b) panel.
                    for nn in range(4):
                        g = 4 * h + nn
                        dst = eT_sb[:, g * 512:(g + 1) * 512]
                        if g % 2 == 0:
                            nc.vector.tensor_copy(dst, psbank[g])
                        else:
                            nc.scalar.copy(dst, psbank[g])
                        nc.gpsimd.dma_start(f_raw[g * T:(g + 1) * T], dst)

            # F_all[(g,j), (c3, s, b)] = exp(e - sigma + fcb) elementwise
            f_all = fpool.tile([P_SCAN, C3 * L * NB], dt.float32)
            nc.scalar.activation(f_all, f_raw,
                                 mybir.ActivationFunctionType.Exp, bias=biasF_sb)
            # chunk-0 s=0 slots: alpha_0 = exp(start_j + fcb_j + e_0[b,j])
            nc.scalar.activation(
                f_all[0:T, 0:NB], f_raw[0:T, 0:NB],
                mybir.ActivationFunctionType.Exp, bias=bias0_sb)
            f_v = f_all.rearrange("p (c3 s b) -> p c3 s b", c3=C3, s=L)

            # ---- scan: chunk c = 8g + c3, 8 steps, two independent
            # half-chains (c3 0-3 | 4-7) that interleave on PE/DVE ----
            def fslice(s, h):
                return f_v[:, 4 * h:4 * h + 4, s, :].unsqueeze(-1).broadcast_to(
                    [P_SCAN, 4, NB, T])

            # interleave the two chains' emission: engines run their program
            # IN ORDER, so A1,B1,A2,B2,... lets chain B's matmul fill the PE
            # while chain A's scale runs on the DVE (and vice versa)
            with tc.tile_pool(name="psq", bufs=4, space="PSUM") as psqpool:
                qcur = []
                for h in range(2):
                    q = qpool.tile([P_SCAN, 4, NB, T], dt.bfloat16, tag=f"q{h}",
                                   name=f"q{h}")
                    ep = epat_sb[:, h * HALF:(h + 1) * HALF]
                    nc.vector.tensor_mul(
                        q, ep.rearrange("p (c3 b i) -> p c3 b i", c3=4, b=NB),
                        fslice(0, h))
                    qcur.append(q)
                for s in range(1, L):
                    for h in range(2):
                        psq = psqpool.tile([P_SCAN, HALF], dt.float32, tag="psq",
                                           name="psq")
                        nc.tensor.matmul(
                            psq, lhsE_sb,
                            qcur[h].rearrange("p c3 b i -> p (c3 b i)"),
                            start=True, stop=True)
                        qn = qpool.tile([P_SCAN, 4, NB, T], dt.bfloat16,
                                        tag=f"q{h}", name=f"qn{h}")
                        nc.vector.tensor_mul(
                            qn, psq.rearrange("p (c3 b i) -> p c3 b i",
                                              c3=4, b=NB),
                            fslice(s, h))
                        qcur[h] = qn
                for h in range(2):
                    # back to f32 for the host
                    qff = fpool.tile([P_SCAN, HALF], dt.float32, tag=f"qf{h}",
                                     name=f"qf{h}")
                    nc.vector.tensor_copy(
                        qff, qcur[h].rearrange("p c3 b i -> p (c3 b i)"))
                    nc.gpsimd.dma_start(q_out[:, h * HALF:(h + 1) * HALF], qff)
            nc.gpsimd.dma_start(eT_out[:, :], eT_sb)

    nc.compile()
    return nc


def _get_nc():
    if "nc" not in _cached:
        _cached["nc"] = _build_nc()
    return _cached["nc"]


def _host_prep(hidden_states, fc_w, fc_b, start_transitions, transitions):
    """Build the 8 per-core input maps."""
    import ml_dtypes
    np_mdt = {"f32": np.float32, "f32r": np.float32,
              "bf16": ml_dtypes.bfloat16,
              "fp8": ml_dtypes.float8_e4m3}[MOVING_DTYPE]

    E = np.exp(transitions.astype(np.float64)).astype(np.float32)     # [T,T]
    # epat[(g,j),(c3,b,i)] = E[i,j], except chunk 0 (g=0, c3=0) slots = 1
    epat = np.tile(E.T[None, :, None, None, :], (NG, 1, C3, NB, 1))   # [g,j,c3,b,i]
    epat[0, :, 0, :, :] = 1.0
    epat = np.ascontiguousarray(epat.reshape(P_SCAN, NFREE), dtype=np.float32)
    # lhsE = blockdiag(E) x8: lhsT[(g,k),(g,j)] = E[k,j]  (bf16 scan matmul)
    lhsE = np.zeros((P_SCAN, P_SCAN), dtype=ml_dtypes.bfloat16)
    for g in range(NG):
        lhsE[g * T:(g + 1) * T, g * T:(g + 1) * T] = E.astype(ml_dtypes.bfloat16)
    fcwT = np.ascontiguousarray(fc_w.T.astype(np_mdt))                # [H,T]
    biasF = np.ascontiguousarray(
        np.tile(fc_b - SIGMA, NG).reshape(P_SCAN, 1), dtype=np.float32)
    bias0 = np.ascontiguousarray(
        (start_transitions + fc_b).reshape(T, 1), dtype=np.float32)

    in_maps = []
    for cid in range(NCORES):
        hc = hidden_states[cid * NB:(cid + 1) * NB]                   # [NB,S,H]
        # t-major token order: col = t*NB + b
        hc = hc.transpose(1, 0, 2).reshape(NTOK, H)
        hTc = np.ascontiguousarray(hc.T.astype(np_mdt))               # [H,4096]
        in_maps.append({
            "hT": hTc, "fcwT": fcwT, "lhsE": lhsE, "epat": epat,
            "biasF": biasF, "bias0": bias0,
        })
    return in_maps


def _host_finish(results, labels, fc_b, start_transitions,
                 end_transitions, transitions):
    """Numerator + chunk-matrix combine, all in f64."""
    labels = labels.astype(np.int64)
    start = start_transitions.astype(np.float64)
    end = end_transitions.astype(np.float64)
    trans = transitions.astype(np.float64)

    # reassemble e [B, S, T] from per-core e^T [9, 4096] (+ fc_b)
    # token order is t-major: col = t*NB + b
    e = np.empty((B, S, T), dtype=np.float64)
    for cid in range(NCORES):
        eT = results[cid]["eT_out"].astype(np.float64)    # [9, 4096]
        e[cid * NB:(cid + 1) * NB] = eT.T.reshape(S, NB, T).transpose(1, 0, 2)
    e += fc_b.astype(np.float64)

    # numerator (mask all-ones fast path)
    emit = np.take_along_axis(e, labels[..., None], axis=-1)[..., 0]
    num = start[labels[:, 0]] + emit[:, 0]
    num = num + (trans[labels[:, :-1], labels[:, 1:]] + emit[:, 1:]).sum(1)
    num = num + end[labels[:, -1]]

    # denominator: combine chunk matrices
    # chunk c = 8g + c3;  Q[(g,j), (c3,b,i)] = P_c[i, j]
    den = np.empty(B)
    for cid in range(NCORES):
        Q = results[cid]["q_out"].astype(np.float64)      # [72, 576]
        Q = Q.reshape(NG, T, C3, NB, T)                   # [g, j, c3, b, i]
        for b in range(NB):
            alpha = Q[0, :, 0, b, 0].copy()  # P_0[0,:] (rows of P_0 all equal)
            corr = 0.0
            for c in range(1, C):
                g, c3 = c // C3, c % C3
                Pc = Q[g, :, c3, b, :].T                  # P_c[i, j] rows i
                alpha = alpha @ Pc
                m = alpha.max()
                alpha /= m
                corr += np.log(m)
            den[cid * NB + b] = np.log((alpha * np.exp(end)).sum()) + corr \
                + (S - 1) * SIGMA
    return np.float32(-(num - den).sum())


def kernel(**inputs):
    hidden_states = np.asarray(inputs["hidden_states"], dtype=np.float32)
    attention_mask = np.asarray(inputs["attention_mask"])
    labels = np.asarray(inputs["labels"])
    fc_w = np.asarray(inputs["fc_w"], dtype=np.float32)
    fc_b = np.asarray(inputs["fc_b"], dtype=np.float32)
    start_transitions = np.asarray(inputs["start_transitions"], dtype=np.float32)
    end_transitions = np.asarray(inputs["end_transitions"], dtype=np.float32)
    transitions = np.asarray(inputs["transitions"], dtype=np.float32)

    if (hidden_states.shape != (B, S, H)) or not np.all(attention_mask != 0):
        return _reference_host(hidden_states, attention_mask, labels, fc_w,
                               fc_b, start_transitions, end_transitions,
                               transitions)

    from concourse.bass_utils import run_bass_kernel_spmd
    nc = _get_nc()
    in_maps = _host_prep(hidden_states, fc_w, fc_b, start_transitions,
                         transitions)
    res = run_bass_kernel_spmd(nc, in_maps, core_ids=list(range(NCORES)))
    _cached["last_res"] = res
    return _host_finish(res.results, labels, fc_b, start_transitions,
                        end_transitions, transitions)


# revision 48
# speedup vs baseline: 1.3361x; 1.0307x over previous
"""BertCrf loss kernel for Trainium2 (8 NeuronCores, SPMD data-parallel).

Strategy
--------
Shapes: B=64, S=512, H=768, T=9 tags.  Loss = -sum_b(num_b - den_b).

The only heavy data is hidden_states [64,512,768] f32 (100 MB) -> the kernel
is memory-bound on streaming it once.  Each of the 8 cores takes 8 sequences.

Phase 1 (device, DMA-bound): emissions e^T [9, 4096] = fc_w @ h^T per core,
streamed from a host-pre-transposed hT [768, 4096] so the contraction dim
lands on partitions.  e^T (without fc_b; host adds it) is DMA'd back to the
host (147 KB/core) for the numerator.

Phase 2 (device): the CRF log-partition recurrence
  alpha_t[j] = logsumexp_k(alpha_{t-1,k} + trans[k,j]) + e_t[j]
is associative in the (log,+) semiring.  In linear space each step is
  P <- P @ (E * f_t[None,:]),  E = exp(trans), f_t = exp(e_t + fc_b - sigma),
so each length-8 chunk's product matrix is computed independently ->
8 seqs x 64 chunks = 512 independent 9x9 matrix chains, all advanced by a
SINGLE sequence of 8 steps: per step one block-diagonal [72,72]x[72,288]
bf16 matmul per half + one [72,288] VectorE scale per half (the two
halves are independent chains whose emission is interleaved so PE and DVE
ping-pong).  The constant shift sigma keeps fp32 in range (chunk
log-range ~ 13; fp32 overflows at 88).

Scan packing (hardcoded): chunk c = 8g + c3 with group g in [0,8),
c3 in [0,8); partitions (g, j) = 72; free index (c3, b, i) = 576.
Q[(g,j), (c3,b,i)] = P_{c,b}[i, j] (state, transposed per pair).
Because the token order is t-major (col = t*8 + b), PSUM bank g of the
emissions matmul (columns [512g, 512g+512)) is exactly group g's
(c3, s, b) factor panel: one [9,512] copy + one Exp builds all 512
chunks' scale factors.  Step s=0 is pure elementwise: Q_1 = Epat * F_0,
with Epat[(g,j),(c3,b,i)] = E[i,j] except 1.0 on chunk 0, whose F slot
holds exp(start_j + fc_b_j + e_0[b,j]) -> rows of P_0 all equal alpha_0.

DMA: 3.15MB/core (hidden cast to fp8-e4m3 on host; the loss is insensitive
because emission errors largely cancel between numerator and denominator,
measured 8e-5 end-to-end) streamed as half-rows over both HWDGE rings,
half 0 first so bank 0-3 copies/F-gathers overlap half-1 streaming.

Host (cheap, exact f64): numerator from labels + e^T; combine the 64
chunk matrices per sequence (tiny 9x9 matvecs) with renormalization;
final logsumexp with end_transitions.  A full numpy fallback handles any
non-all-ones attention mask (the benchmark's mask is always ones).
"""

import numpy as np

# ---- problem constants (hardcoded per the task contract) ----
B, S, H, T = 64, 512, 768, 9
NCORES = 8
NB = B // NCORES          # 8 local sequences per core
NTOK = NB * S             # 4096 tokens per core
L = 8                     # chunk length (timesteps per chunk)
C = S // L                # 64 chunks
NG = 8                    # partition groups; chunk c = 8g + c3
C3 = 8                    # chunks per group
P_SCAN = NG * T           # 72 scan partitions
NFREE = C3 * NB * T       # 576 scan free columns (c3, b, i)
SIGMA = 0.8               # linear-space shift (range control)
KT = H // 128             # 6 contraction tiles

# token order is t-major: column index = t*NB + b.  Then PSUM bank g of the
# emissions matmul (columns [512g, 512g+512)) is exactly scan group g's
# (c3, s, b) panel: col = 512g + 64*c3 + 8*s + b.

MOVING_DTYPE = "fp8"      # "f32" | "f32r" | "bf16" | "fp8"  (hidden/fc_w dtype)

_cached = {}


def _np_logsumexp(x, axis):
    m = np.max(x, axis=axis, keepdims=True)
    return (m + np.log(np.sum(np.exp(x - m), axis=axis, keepdims=True))).squeeze(axis)


def _reference_host(hidden_states, attention_mask, labels, fc_w, fc_b,
                    start_transitions, end_transitions, transitions):
    """Exact numpy port of the reference (f64) - fallback for unusual inputs."""
    e = (hidden_states.astype(np.float64) @ fc_w.T.astype(np.float64)) + fc_b
    mask = attention_mask.astype(bool)
    maskf = mask.astype(np.float64)
    labels = labels.astype(np.int64)
    b_idx = np.arange(e.shape[0])

    emit = np.take_along_axis(e, labels[..., None], axis=-1)[..., 0]
    trans_sc = transitions[labels[:, :-1], labels[:, 1:]].astype(np.float64)
    num = start_transitions[labels[:, 0]].astype(np.float64) + emit[:, 0]
    num = num + ((trans_sc + emit[:, 1:]) * maskf[:, 1:]).sum(1)
    last_idx = mask.astype(np.int64).sum(1) - 1
    num = num + end_transitions[labels[b_idx, last_idx]]

    alpha = start_transitions[None, :].astype(np.float64) + e[:, 0]
    for t in range(1, e.shape[1]):
        nxt = _np_logsumexp(alpha[:, :, None] + transitions[None].astype(np.float64)
                            + e[:, t][:, None, :], axis=1)
        alpha = np.where(mask[:, t][:, None], nxt, alpha)
    den = _np_logsumexp(alpha + end_transitions[None, :].astype(np.float64), axis=1)
    return np.float32(-(num - den).sum())


def _build_nc():
    """Build the per-core Bass program (same program on all 8 cores)."""
    import concourse.bacc as bacc
    import concourse.mybir as mybir
    import concourse.tile as tile

    dt = mybir.dt
    mdt = {"f32": dt.float32, "f32r": dt.float32, "bf16": dt.bfloat16,
           "fp8": dt.float8e4}[MOVING_DTYPE]

    # Bacc (not raw Bass): its compile() pass legalizes multi-wait sync_info
    # into what this walrus build's per-instruction wait slots accept.
    nc = bacc.Bacc("TRN2", target_bir_lowering=False, debug=False)

    hT = nc.dram_tensor("hT", [H, NTOK], mdt, kind="ExternalInput")
    fcwT = nc.dram_tensor("fcwT", [H, T], mdt, kind="ExternalInput")
    lhsE = nc.dram_tensor("lhsE", [P_SCAN, P_SCAN], dt.bfloat16, kind="ExternalInput")
    epat = nc.dram_tensor("epat", [P_SCAN, NFREE], dt.float32, kind="ExternalInput")
    biasF = nc.dram_tensor("biasF", [P_SCAN, 1], dt.float32, kind="ExternalInput")
    bias0 = nc.dram_tensor("bias0", [T, 1], dt.float32, kind="ExternalInput")
    eT_out = nc.dram_tensor("eT_out", [T, NTOK], dt.float32, kind="ExternalOutput")
    q_out = nc.dram_tensor("q_out", [P_SCAN, NFREE], dt.float32,
                           kind="ExternalOutput")

    f32r = dt.float32r
    HALF = NFREE // 2          # 288 free columns per scan half-chain

    with tile.TileContext(nc) as tc:
        with (
            tc.tile_pool(name="const", bufs=1) as cpool,
            tc.tile_pool(name="hbuf", bufs=1) as hpool,
            tc.tile_pool(name="fbuf", bufs=1) as fpool,
            tc.tile_pool(name="scan", bufs=2) as qpool,
        ):
            # ---- phase 1 loads: 12 half-row 512KB DMAs on the two HWDGE
            # rings, HALF-MAJOR order: columns [0,2048) land first so banks
            # 0-3 finish (and their copies + F gathers run) while columns
            # [2048,4096) are still streaming ----
            hT_r = hT.rearrange("(kt p) n -> kt p n", p=128)
            fcw_sb = cpool.tile([128, KT, T], mdt)
            nc.gpsimd.dma_start(fcw_sb, fcwT.rearrange("(kt p) m -> p kt m", p=128))
            lhsE_sb = cpool.tile([P_SCAN, P_SCAN], dt.bfloat16)
            nc.gpsimd.dma_start(lhsE_sb, lhsE[:, :])
            epat_sb = cpool.tile([P_SCAN, NFREE], dt.float32)
            nc.gpsimd.dma_start(epat_sb, epat[:, :])
            biasF_sb = cpool.tile([P_SCAN, 1], dt.float32)
            nc.gpsimd.dma_start(biasF_sb, biasF[:, :])
            bias0_sb = cpool.tile([T, 1], dt.float32)
            nc.gpsimd.dma_start(bias0_sb, bias0[:, :])

            # 12 half-row DMAs on the two HWDGE rings, HALF-MAJOR order:
            # columns [0,2048) land first so banks 0-3 finish (and their
            # copies + F gathers run) while columns [2048,4096) stream
            HC = NTOK // 2
            htiles = [[None] * 2 for _ in range(KT)]
            for h in range(2):
                for kt in range(KT):
                    ht = hpool.tile([128, HC], mdt, tag=f"ht{kt}_{h}",
                                    name=f"ht{kt}_{h}")
                    eng = nc.sync if kt % 2 == 0 else nc.scalar
                    eng.dma_start(ht, hT_r[kt, :, h * HC:(h + 1) * HC])
                    htiles[kt][h] = ht

            f_raw = fpool.tile([P_SCAN, C3 * L * NB], dt.float32)
            eT_sb = fpool.tile([T, NTOK], dt.float32)
            with tc.tile_pool(name="psum1", bufs=1, space="PSUM") as pspool:
                psbank = [pspool.tile([T, 512], dt.float32, tag=f"psb{i}",
                                      name=f"psb{i}")
                          for i in range(8)]
                for h in range(2):
                    for kt in range(KT):
                        for nn in range(4):
                            n = 4 * h + nn
                            lw = fcw_sb[:, kt, :]
                            rh = htiles[kt][h][:, nn * 512:(nn + 1) * 512]
                            if MOVING_DTYPE == "f32r":
                                lw = lw.bitcast(f32r)
                                rh = rh.bitcast(f32r)
                            nc.tensor.matmul(psbank[n], lw, rh,
                                             start=(kt == 0), stop=(kt == KT - 1))
                    # banks of this half are done: PSUM->SBUF copy (DMA can't
                    # read PSUM; alternate DVE/ACT), then the F gather for
                    # scan group g on the SWDGE queue (rings are still
                    # streaming hT).  Column block [512g, 512(g+1)) of e^T is
                    # exactly group g's (c3, s, b) panel.
                    for nn in range(4):
                        g = 4 * h + nn
                        dst = eT_sb[:, g * 512:(g + 1) * 512]
                        if g % 2 == 0:
                            nc.vector.tensor_copy(dst, psbank[g])
                        else:
                            nc.scalar.copy(dst, psbank[g])
                        # h0 gathers on gpsimd; h1 on the sync ring (idle
                        # by the time the h1 copies finish) -> ~2x faster F
                        feng = nc.gpsimd if h == 0 else nc.sync
                        feng.dma_start(f_raw[g * T:(g + 1) * T], dst)

            # F_all[(g,j), (c3, s, b)] = exp(e - sigma + fcb) elementwise
            f_all = fpool.tile([P_SCAN, C3 * L * NB], dt.float32)
            nc.scalar.activation(f_all, f_raw,
                                 mybir.ActivationFunctionType.Exp, bias=biasF_sb)
            # chunk-0 s=0 slots: alpha_0 = exp(start_j + fcb_j + e_0[b,j])
            nc.scalar.activation(
                f_all[0:T, 0:NB], f_raw[0:T, 0:NB],
                mybir.ActivationFunctionType.Exp, bias=bias0_sb)
            f_v = f_all.rearrange("p (c3 s b) -> p c3 s b", c3=C3, s=L)

            # ---- scan: chunk c = 8g + c3, 8 steps, two independent
            # half-chains (c3 0-3 | 4-7) that interleave on PE/DVE ----
            def fslice(s, h):
                return f_v[:, 4 * h:4 * h + 4, s, :].unsqueeze(-1).broadcast_to(
                    [P_SCAN, 4, NB, T])

            # interleave the two chains' emission: engines run their program
            # IN ORDER, so A1,B1,A2,B2,... lets chain B's matmul fill the PE
            # while chain A's scale runs on the DVE (and vice versa)
            with tc.tile_pool(name="psq", bufs=4, space="PSUM") as psqpool:
                qcur = []
                for h in range(2):
                    q = qpool.tile([P_SCAN, 4, NB, T], dt.bfloat16, tag=f"q{h}",
                                   name=f"q{h}")
                    ep = epat_sb[:, h * HALF:(h + 1) * HALF]
                    nc.vector.tensor_mul(
                        q, ep.rearrange("p (c3 b i) -> p c3 b i", c3=4, b=NB),
                        fslice(0, h))
                    qcur.append(q)
                for s in range(1, L):
                    for h in range(2):
                        psq = psqpool.tile([P_SCAN, HALF], dt.float32, tag="psq",
                                           name="psq")
                        nc.tensor.matmul(
                            psq, lhsE_sb,
                            qcur[h].rearrange("p c3 b i -> p (c3 b i)"),
                            start=True, stop=True)
                        # final step lands in f32 so the host output needs no
                        # extra cast pass
                        qdt = dt.float32 if s == L - 1 else dt.bfloat16
                        qtag = f"qf{h}" if s == L - 1 else f"q{h}"
                        qn = qpool.tile([P_SCAN, 4, NB, T], qdt,
                                        tag=qtag, name=f"qn{h}_{s}")
                        nc.vector.tensor_mul(
                            qn, psq.rearrange("p (c3 b i) -> p c3 b i",
                                              c3=4, b=NB),
                            fslice(s, h))
                        qcur[h] = qn
                for h in range(2):
                    nc.sync.dma_start(
                        q_out[:, h * HALF:(h + 1) * HALF],
                        qcur[h].rearrange("p c3 b i -> p (c3 b i)"))
            nc.gpsimd.dma_start(eT_out[:, :], eT_sb)

    nc.compile()
    return nc


def _get_nc():
    if "nc" not in _cached:
        _cached["nc"] = _build_nc()
    return _cached["nc"]


def _host_prep(hidden_states, fc_w, fc_b, start_transitions, transitions):
    """Build the 8 per-core input maps."""
    import ml_dtypes
    np_mdt = {"f32": np.float32, "f32r": np.float32,
              "bf16": ml_dtypes.bfloat16,
              "fp8": ml_dtypes.float8_e4m3}[MOVING_DTYPE]

    E = np.exp(transitions.astype(np.float64)).astype(np.float32)     # [T,T]
    # epat[(g,j),(c3,b,i)] = E[i,j], except chunk 0 (g=0, c3=0) slots = 1
    epat = np.tile(E.T[None, :, None, None, :], (NG, 1, C3, NB, 1))   # [g,j,c3,b,i]
    epat[0, :, 0, :, :] = 1.0
    epat = np.ascontiguousarray(epat.reshape(P_SCAN, NFREE), dtype=np.float32)
    # lhsE = blockdiag(E) x8: lhsT[(g,k),(g,j)] = E[k,j]  (bf16 scan matmul)
    lhsE = np.zeros((P_SCAN, P_SCAN), dtype=ml_dtypes.bfloat16)
    for g in range(NG):
        lhsE[g * T:(g + 1) * T, g * T:(g + 1) * T] = E.astype(ml_dtypes.bfloat16)
    fcwT = np.ascontiguousarray(fc_w.T.astype(np_mdt))                # [H,T]
    biasF = np.ascontiguousarray(
        np.tile(fc_b - SIGMA, NG).reshape(P_SCAN, 1), dtype=np.float32)
    bias0 = np.ascontiguousarray(
        (start_transitions + fc_b).reshape(T, 1), dtype=np.float32)

    in_maps = []
    for cid in range(NCORES):
        hc = hidden_states[cid * NB:(cid + 1) * NB]                   # [NB,S,H]
        # t-major token order: col = t*NB + b
        hc = hc.transpose(1, 0, 2).reshape(NTOK, H)
        hTc = np.ascontiguousarray(hc.T.astype(np_mdt))               # [H,4096]
        in_maps.append({
            "hT": hTc, "fcwT": fcwT, "lhsE": lhsE, "epat": epat,
            "biasF": biasF, "bias0": bias0,
        })
    return in_maps


def _host_finish(results, labels, fc_b, start_transitions,
                 end_transitions, transitions):
    """Numerator + chunk-matrix combine, all in f64."""
    labels = labels.astype(np.int64)
    start = start_transitions.astype(np.float64)
    end = end_transitions.astype(np.float64)
    trans = transitions.astype(np.float64)

    # reassemble e [B, S, T] from per-core e^T [9, 4096] (+ fc_b)
    # token order is t-major: col = t*NB + b
    e = np.empty((B, S, T), dtype=np.float64)
    for cid in range(NCORES):
        eT = results[cid]["eT_out"].astype(np.float64)    # [9, 4096]
        e[cid * NB:(cid + 1) * NB] = eT.T.reshape(S, NB, T).transpose(1, 0, 2)
    e += fc_b.astype(np.float64)

    # numerator (mask all-ones fast path)
    emit = np.take_along_axis(e, labels[..., None], axis=-1)[..., 0]
    num = start[labels[:, 0]] + emit[:, 0]
    num = num + (trans[labels[:, :-1], labels[:, 1:]] + emit[:, 1:]).sum(1)
    num = num + end[labels[:, -1]]

    # denominator: combine chunk matrices
    # chunk c = 8g + c3;  Q[(g,j), (c3,b,i)] = P_c[i, j]
    den = np.empty(B)
    for cid in range(NCORES):
        Q = results[cid]["q_out"].astype(np.float64)      # [72, 576]
        Q = Q.reshape(NG, T, C3, NB, T)                   # [g, j, c3, b, i]
        for b in range(NB):
            alpha = Q[0, :, 0, b, 0].copy()  # P_0[0,:] (rows of P_0 all equal)
            corr = 0.0
            for c in range(1, C):
                g, c3 = c // C3, c % C3
                Pc = Q[g, :, c3, b, :].T                  # P_c[i, j] rows i
                alpha = alpha @ Pc
                m = alpha.max()
                alpha /= m
                corr += np.log(m)
            den[cid * NB + b] = np.log((alpha * np.exp(end)).sum()) + corr \
                + (S - 1) * SIGMA
    return np.float32(-(num - den).sum())


def kernel(**inputs):
    hidden_states = np.asarray(inputs["hidden_states"], dtype=np.float32)
    attention_mask = np.asarray(inputs["attention_mask"])
    labels = np.asarray(inputs["labels"])
    fc_w = np.asarray(inputs["fc_w"], dtype=np.float32)
    fc_b = np.asarray(inputs["fc_b"], dtype=np.float32)
    start_transitions = np.asarray(inputs["start_transitions"], dtype=np.float32)
    end_transitions = np.asarray(inputs["end_transitions"], dtype=np.float32)
    transitions = np.asarray(inputs["transitions"], dtype=np.float32)

    if (hidden_states.shape != (B, S, H)) or not np.all(attention_mask != 0):
        return _reference_host(hidden_states, attention_mask, labels, fc_w,
                               fc_b, start_transitions, end_transitions,
                               transitions)

    from concourse.bass_utils import run_bass_kernel_spmd
    nc = _get_nc()
    in_maps = _host_prep(hidden_states, fc_w, fc_b, start_transitions,
                         transitions)
    res = run_bass_kernel_spmd(nc, in_maps, core_ids=list(range(NCORES)))
    _cached["last_res"] = res
    return _host_finish(res.results, labels, fc_b, start_transitions,
                        end_transitions, transitions)


# revision 49
# speedup vs baseline: 1.3594x; 1.0174x over previous
"""BertCrf loss kernel for Trainium2 (8 NeuronCores, SPMD data-parallel).

Strategy
--------
Shapes: B=64, S=512, H=768, T=9 tags.  Loss = -sum_b(num_b - den_b).

The only heavy data is hidden_states [64,512,768] f32 (100 MB) -> the kernel
is memory-bound on streaming it once.  Each of the 8 cores takes 8 sequences.

Phase 1 (device, DMA-bound): emissions e^T [9, 4096] = fc_w @ h^T per core,
streamed from a host-pre-transposed hT [768, 4096] so the contraction dim
lands on partitions.  e^T (without fc_b; host adds it) is DMA'd back to the
host (147 KB/core) for the numerator.

Phase 2 (device): the CRF log-partition recurrence
  alpha_t[j] = logsumexp_k(alpha_{t-1,k} + trans[k,j]) + e_t[j]
is associative in the (log,+) semiring.  In linear space each step is
  P <- P @ (E * f_t[None,:]),  E = exp(trans), f_t = exp(e_t + fc_b - sigma),
so each length-8 chunk's product matrix is computed independently ->
8 seqs x 64 chunks = 512 independent 9x9 matrix chains, all advanced by a
SINGLE sequence of 8 steps: per step one block-diagonal [72,72]x[72,288]
bf16 matmul per half + one [72,288] VectorE scale per half (the two
halves are independent chains whose emission is interleaved so PE and DVE
ping-pong).  The constant shift sigma keeps fp32 in range (chunk
log-range ~ 13; fp32 overflows at 88).

Scan packing (hardcoded): chunk c = 8g + c3 with group g in [0,8),
c3 in [0,8); partitions (g, j) = 72; free index (c3, b, i) = 576.
Q[(g,j), (c3,b,i)] = P_{c,b}[i, j] (state, transposed per pair).
Because the token order is t-major (col = t*8 + b), PSUM bank g of the
emissions matmul (columns [512g, 512g+512)) is exactly group g's
(c3, s, b) factor panel: one [9,512] copy + one Exp builds all 512
chunks' scale factors.  Step s=0 is pure elementwise: Q_1 = Epat * F_0,
with Epat[(g,j),(c3,b,i)] = E[i,j] except 1.0 on chunk 0, whose F slot
holds exp(start_j + fc_b_j + e_0[b,j]) -> rows of P_0 all equal alpha_0.

DMA: 3.15MB/core (hidden cast to fp8-e4m3 on host; the loss is insensitive
because emission errors largely cancel between numerator and denominator,
measured 8e-5 end-to-end) streamed as half-rows over both HWDGE rings,
half 0 first so bank 0-3 copies/F-gathers overlap half-1 streaming.

Host (cheap, exact f64): numerator from labels + e^T; combine the 64
chunk matrices per sequence (tiny 9x9 matvecs) with renormalization;
final logsumexp with end_transitions.  A full numpy fallback handles any
non-all-ones attention mask (the benchmark's mask is always ones).
"""

import numpy as np

# ---- problem constants (hardcoded per the task contract) ----
B, S, H, T = 64, 512, 768, 9
NCORES = 8
NB = B // NCORES          # 8 local sequences per core
NTOK = NB * S             # 4096 tokens per core
L = 8                     # chunk length (timesteps per chunk)
C = S // L                # 64 chunks
NG = 8                    # partition groups; chunk c = 8g + c3
C3 = 8                    # chunks per group
P_SCAN = NG * T           # 72 scan partitions
NFREE = C3 * NB * T       # 576 scan free columns (c3, b, i)
SIGMA = 0.8               # linear-space shift (range control)
KT = H // 128             # 6 contraction tiles

# token order is t-major: column index = t*NB + b.  Then PSUM bank g of the
# emissions matmul (columns [512g, 512g+512)) is exactly scan group g's
# (c3, s, b) panel: col = 512g + 64*c3 + 8*s + b.

MOVING_DTYPE = "fp8"      # "f32" | "f32r" | "bf16" | "fp8"  (hidden/fc_w dtype)

_cached = {}


def _np_logsumexp(x, axis):
    m = np.max(x, axis=axis, keepdims=True)
    return (m + np.log(np.sum(np.exp(x - m), axis=axis, keepdims=True))).squeeze(axis)


def _reference_host(hidden_states, attention_mask, labels, fc_w, fc_b,
                    start_transitions, end_transitions, transitions):
    """Exact numpy port of the reference (f64) - fallback for unusual inputs."""
    e = (hidden_states.astype(np.float64) @ fc_w.T.astype(np.float64)) + fc_b
    mask = attention_mask.astype(bool)
    maskf = mask.astype(np.float64)
    labels = labels.astype(np.int64)
    b_idx = np.arange(e.shape[0])

    emit = np.take_along_axis(e, labels[..., None], axis=-1)[..., 0]
    trans_sc = transitions[labels[:, :-1], labels[:, 1:]].astype(np.float64)
    num = start_transitions[labels[:, 0]].astype(np.float64) + emit[:, 0]
    num = num + ((trans_sc + emit[:, 1:]) * maskf[:, 1:]).sum(1)
    last_idx = mask.astype(np.int64).sum(1) - 1
    num = num + end_transitions[labels[b_idx, last_idx]]

    alpha = start_transitions[None, :].astype(np.float64) + e[:, 0]
    for t in range(1, e.shape[1]):
        nxt = _np_logsumexp(alpha[:, :, None] + transitions[None].astype(np.float64)
                            + e[:, t][:, None, :], axis=1)
        alpha = np.where(mask[:, t][:, None], nxt, alpha)
    den = _np_logsumexp(alpha + end_transitions[None, :].astype(np.float64), axis=1)
    return np.float32(-(num - den).sum())


def _build_nc():
    """Build the per-core Bass program (same program on all 8 cores)."""
    import concourse.bacc as bacc
    import concourse.mybir as mybir
    import concourse.tile as tile

    dt = mybir.dt
    mdt = {"f32": dt.float32, "f32r": dt.float32, "bf16": dt.bfloat16,
           "fp8": dt.float8e4}[MOVING_DTYPE]

    # Bacc (not raw Bass): its compile() pass legalizes multi-wait sync_info
    # into what this walrus build's per-instruction wait slots accept.
    nc = bacc.Bacc("TRN2", target_bir_lowering=False, debug=False)

    hT = nc.dram_tensor("hT", [H, NTOK], mdt, kind="ExternalInput")
    fcwT = nc.dram_tensor("fcwT", [H, T], mdt, kind="ExternalInput")
    lhsE = nc.dram_tensor("lhsE", [P_SCAN, P_SCAN], dt.bfloat16, kind="ExternalInput")
    epat = nc.dram_tensor("epat", [P_SCAN, NFREE], dt.float32, kind="ExternalInput")
    biasF = nc.dram_tensor("biasF", [P_SCAN, 1], dt.float32, kind="ExternalInput")
    bias0 = nc.dram_tensor("bias0", [T, 1], dt.float32, kind="ExternalInput")
    eT_out = nc.dram_tensor("eT_out", [T, NTOK], dt.float32, kind="ExternalOutput")
    q_out = nc.dram_tensor("q_out", [P_SCAN, NFREE], dt.float32,
                           kind="ExternalOutput")

    f32r = dt.float32r
    HALF = NFREE // 2          # 288 free columns per scan half-chain

    with tile.TileContext(nc) as tc:
        with (
            tc.tile_pool(name="const", bufs=1) as cpool,
            tc.tile_pool(name="hbuf", bufs=1) as hpool,
            tc.tile_pool(name="fbuf", bufs=1) as fpool,
            tc.tile_pool(name="scan", bufs=2) as qpool,
        ):
            # ---- phase 1 loads: 12 half-row 512KB DMAs on the two HWDGE
            # rings, HALF-MAJOR order: columns [0,2048) land first so banks
            # 0-3 finish (and their copies + F gathers run) while columns
            # [2048,4096) are still streaming ----
            hT_r = hT.rearrange("(kt p) n -> kt p n", p=128)
            # DoubleRow fp8 weights: [128, ktpair, 2, 16] with M padded 9->16
            # (the 2-ktile step must be 16-byte aligned); pad rows produce
            # junk PSUM partitions 9-15 that nothing reads.
            fcw_sb = cpool.tile([128, KT // 2, 2, 16], mdt)
            nc.gpsimd.memset(fcw_sb, 0.0)
            nc.gpsimd.dma_start(
                fcw_sb[:, :, :, 0:T],
                fcwT.rearrange("(ktp two p) m -> p ktp two m", two=2, p=128))
            lhsE_sb = cpool.tile([P_SCAN, P_SCAN], dt.bfloat16)
            nc.gpsimd.dma_start(lhsE_sb, lhsE[:, :])
            epat_sb = cpool.tile([P_SCAN, NFREE], dt.float32)
            nc.gpsimd.dma_start(epat_sb, epat[:, :])
            biasF_sb = cpool.tile([P_SCAN, 1], dt.float32)
            nc.gpsimd.dma_start(biasF_sb, biasF[:, :])
            bias0_sb = cpool.tile([T, 1], dt.float32)
            nc.gpsimd.dma_start(bias0_sb, bias0[:, :])

            # 12 half-row DMAs on the two HWDGE rings, HALF-MAJOR order:
            # columns [0,2048) land first so banks 0-3 finish (and their
            # copies + F gathers run) while columns [2048,4096) stream
            HC = NTOK // 2
            htiles = [[None] * 2 for _ in range(KT // 2)]
            for h in range(2):
                for ktp in range(KT // 2):
                    ht = hpool.tile([128, 2, HC], mdt, tag=f"ht{ktp}_{h}",
                                    name=f"ht{ktp}_{h}")
                    for two in range(2):
                        kt = 2 * ktp + two
                        eng = nc.sync if kt % 2 == 0 else nc.scalar
                        eng.dma_start(ht[:, two, :],
                                      hT_r[kt, :, h * HC:(h + 1) * HC])
                    htiles[ktp][h] = ht

            f_raw = fpool.tile([P_SCAN, C3 * L * NB], dt.float32)
            eT_sb = fpool.tile([T, NTOK], dt.float32)
            with tc.tile_pool(name="psum1", bufs=1, space="PSUM") as pspool:
                psbank = [pspool.tile([16, 512], dt.float32, tag=f"psb{i}",
                                      name=f"psb{i}")
                          for i in range(8)]
                for h in range(2):
                    for ktp in range(KT // 2):
                        for nn in range(4):
                            n = 4 * h + nn
                            lw = fcw_sb[:, ktp]
                            rh = htiles[ktp][h][:, :, nn * 512:(nn + 1) * 512]
                            nc.tensor.matmul(
                                psbank[n], lw, rh,
                                start=(ktp == 0), stop=(ktp == KT // 2 - 1),
                                perf_mode=mybir.MatmulPerfMode.DoubleRow)
                    # banks of this half are done: PSUM->SBUF copy (DMA can't
                    # read PSUM; alternate DVE/ACT), then the F gather for
                    # scan group g on the SWDGE queue (rings are still
                    # streaming hT).  Column block [512g, 512(g+1)) of e^T is
                    # exactly group g's (c3, s, b) panel.
                    for nn in range(4):
                        g = 4 * h + nn
                        dst = eT_sb[:, g * 512:(g + 1) * 512]
                        if g % 2 == 0:
                            nc.vector.tensor_copy(dst, psbank[g][0:T, :])
                        else:
                            nc.scalar.copy(dst, psbank[g][0:T, :])
                        # h0 gathers on gpsimd; h1 on the sync ring (idle
                        # by the time the h1 copies finish) -> ~2x faster F
                        feng = nc.gpsimd if h == 0 else nc.sync
                        feng.dma_start(f_raw[g * T:(g + 1) * T], dst)

            # F_all[(g,j), (c3, s, b)] = exp(e - sigma + fcb) elementwise
            f_all = fpool.tile([P_SCAN, C3 * L * NB], dt.float32)
            nc.scalar.activation(f_all, f_raw,
                                 mybir.ActivationFunctionType.Exp, bias=biasF_sb)
            # chunk-0 s=0 slots: alpha_0 = exp(start_j + fcb_j + e_0[b,j])
            nc.scalar.activation(
                f_all[0:T, 0:NB], f_raw[0:T, 0:NB],
                mybir.ActivationFunctionType.Exp, bias=bias0_sb)
            f_v = f_all.rearrange("p (c3 s b) -> p c3 s b", c3=C3, s=L)

            # ---- scan: chunk c = 8g + c3, 8 steps, two independent
            # half-chains (c3 0-3 | 4-7) that interleave on PE/DVE ----
            def fslice(s, h):
                return f_v[:, 4 * h:4 * h + 4, s, :].unsqueeze(-1).broadcast_to(
                    [P_SCAN, 4, NB, T])

            # interleave the two chains' emission: engines run their program
            # IN ORDER, so A1,B1,A2,B2,... lets chain B's matmul fill the PE
            # while chain A's scale runs on the DVE (and vice versa)
            with tc.tile_pool(name="psq", bufs=4, space="PSUM") as psqpool:
                qcur = []
                for h in range(2):
                    q = qpool.tile([P_SCAN, 4, NB, T], dt.bfloat16, tag=f"q{h}",
                                   name=f"q{h}")
                    ep = epat_sb[:, h * HALF:(h + 1) * HALF]
                    nc.vector.tensor_mul(
                        q, ep.rearrange("p (c3 b i) -> p c3 b i", c3=4, b=NB),
                        fslice(0, h))
                    qcur.append(q)
                for s in range(1, L):
                    for h in range(2):
                        psq = psqpool.tile([P_SCAN, HALF], dt.float32, tag="psq",
                                           name="psq")
                        nc.tensor.matmul(
                            psq, lhsE_sb,
                            qcur[h].rearrange("p c3 b i -> p (c3 b i)"),
                            start=True, stop=True)
                        # final step lands in f32 so the host output needs no
                        # extra cast pass
                        qdt = dt.float32 if s == L - 1 else dt.bfloat16
                        qtag = f"qf{h}" if s == L - 1 else f"q{h}"
                        qn = qpool.tile([P_SCAN, 4, NB, T], qdt,
                                        tag=qtag, name=f"qn{h}_{s}")
                        nc.vector.tensor_mul(
                            qn, psq.rearrange("p (c3 b i) -> p c3 b i",
                                              c3=4, b=NB),
                            fslice(s, h))
                        qcur[h] = qn
                for h in range(2):
                    nc.sync.dma_start(
                        q_out[:, h * HALF:(h + 1) * HALF],
                        qcur[h].rearrange("p c3 b i -> p (c3 b i)"))
            nc.gpsimd.dma_start(eT_out[:, :], eT_sb)

    nc.compile()
    return nc


def _get_nc():
    if "nc" not in _cached:
        _cached["nc"] = _build_nc()
    return _cached["nc"]


def _host_prep(hidden_states, fc_w, fc_b, start_transitions, transitions):
    """Build the 8 per-core input maps."""
    import ml_dtypes
    np_mdt = {"f32": np.float32, "f32r": np.float32,
              "bf16": ml_dtypes.bfloat16,
              "fp8": ml_dtypes.float8_e4m3}[MOVING_DTYPE]

    E = np.exp(transitions.astype(np.float64)).astype(np.float32)     # [T,T]
    # epat[(g,j),(c3,b,i)] = E[i,j], except chunk 0 (g=0, c3=0) slots = 1
    epat = np.tile(E.T[None, :, None, None, :], (NG, 1, C3, NB, 1))   # [g,j,c3,b,i]
    epat[0, :, 0, :, :] = 1.0
    epat = np.ascontiguousarray(epat.reshape(P_SCAN, NFREE), dtype=np.float32)
    # lhsE = blockdiag(E) x8: lhsT[(g,k),(g,j)] = E[k,j]  (bf16 scan matmul)
    lhsE = np.zeros((P_SCAN, P_SCAN), dtype=ml_dtypes.bfloat16)
    for g in range(NG):
        lhsE[g * T:(g + 1) * T, g * T:(g + 1) * T] = E.astype(ml_dtypes.bfloat16)
    fcwT = np.ascontiguousarray(fc_w.T.astype(np_mdt))                # [H,T]
    biasF = np.ascontiguousarray(
        np.tile(fc_b - SIGMA, NG).reshape(P_SCAN, 1), dtype=np.float32)
    bias0 = np.ascontiguousarray(
        (start_transitions + fc_b).reshape(T, 1), dtype=np.float32)

    in_maps = []
    for cid in range(NCORES):
        hc = hidden_states[cid * NB:(cid + 1) * NB]                   # [NB,S,H]
        # t-major token order: col = t*NB + b
        hc = hc.transpose(1, 0, 2).reshape(NTOK, H)
        hTc = np.ascontiguousarray(hc.T.astype(np_mdt))               # [H,4096]
        in_maps.append({
            "hT": hTc, "fcwT": fcwT, "lhsE": lhsE, "epat": epat,
            "biasF": biasF, "bias0": bias0,
        })
    return in_maps


def _host_finish(results, labels, fc_b, start_transitions,
                 end_transitions, transitions):
    """Numerator + chunk-matrix combine, all in f64."""
    labels = labels.astype(np.int64)
    start = start_transitions.astype(np.float64)
    end = end_transitions.astype(np.float64)
    trans = transitions.astype(np.float64)

    # reassemble e [B, S, T] from per-core e^T [9, 4096] (+ fc_b)
    # token order is t-major: col = t*NB + b
    e = np.empty((B, S, T), dtype=np.float64)
    for cid in range(NCORES):
        eT = results[cid]["eT_out"].astype(np.float64)    # [9, 4096]
        e[cid * NB:(cid + 1) * NB] = eT.T.reshape(S, NB, T).transpose(1, 0, 2)
    e += fc_b.astype(np.float64)

    # numerator (mask all-ones fast path)
    emit = np.take_along_axis(e, labels[..., None], axis=-1)[..., 0]
    num = start[labels[:, 0]] + emit[:, 0]
    num = num + (trans[labels[:, :-1], labels[:, 1:]] + emit[:, 1:]).sum(1)
    num = num + end[labels[:, -1]]

    # denominator: combine chunk matrices
    # chunk c = 8g + c3;  Q[(g,j), (c3,b,i)] = P_c[i, j]
    den = np.empty(B)
    for cid in range(NCORES):
        Q = results[cid]["q_out"].astype(np.float64)      # [72, 576]
        Q = Q.reshape(NG, T, C3, NB, T)                   # [g, j, c3, b, i]
        for b in range(NB):
            alpha = Q[0, :, 0, b, 0].copy()  # P_0[0,:] (rows of P_0 all equal)
            corr = 0.0
            for c in range(1, C):
                g, c3 = c // C3, c % C3
                Pc = Q[g, :, c3, b, :].T                  # P_c[i, j] rows i
                alpha = alpha @ Pc
                m = alpha.max()
                alpha /= m
                corr += np.log(m)
            den[cid * NB + b] = np.log((alpha * np.exp(end)).sum()) + corr \
                + (S - 1) * SIGMA
    return np.float32(-(num - den).sum())


def kernel(**inputs):
    hidden_states = np.asarray(inputs["hidden_states"], dtype=np.float32)
    attention_mask = np.asarray(inputs["attention_mask"])
    labels = np.asarray(inputs["labels"])
    fc_w = np.asarray(inputs["fc_w"], dtype=np.float32)
    fc_b = np.asarray(inputs["fc_b"], dtype=np.float32)
    start_transitions = np.asarray(inputs["start_transitions"], dtype=np.float32)
    end_transitions = np.asarray(inputs["end_transitions"], dtype=np.float32)
    transitions = np.asarray(inputs["transitions"], dtype=np.float32)

    if (hidden_states.shape != (B, S, H)) or not np.all(attention_mask != 0):
        return _reference_host(hidden_states, attention_mask, labels, fc_w,
                               fc_b, start_transitions, end_transitions,
                               transitions)

    from concourse.bass_utils import run_bass_kernel_spmd
    nc = _get_nc()
    in_maps = _host_prep(hidden_states, fc_w, fc_b, start_transitions,
                         transitions)
    res = run_bass_kernel_spmd(nc, in_maps, core_ids=list(range(NCORES)))
    _cached["last_res"] = res
    return _host_finish(res.results, labels, fc_b, start_transitions,
                        end_transitions, transitions)
